# revision 26
# baseline (speedup 1.0000x reference)
"""Trainium2 Bass kernel for the 21-joint hand-graph message-passing MLP.

Math (per sample b, per target joint t with neighbor list S_t of length n):
    g   = concat(x[b, S_t[0]], ..., x[b, S_t[n-1]])          # [n*64]
    h1  = relu(g @ W1_t + b1_t)                              # [128]
    h2  = relu(h1 @ W2_t + b2_t)                             # [128]
    out[b, t] = h2 @ W3_t + b3_t                             # [64]

Strategy (pure data parallel over 8 NeuronCores, B=65536 -> 8192/core):
  - x is cast-DMA'd (SWDGE fp32->bf16) straight into SBUF batch-major, then
    PE-transposed ([128,128] is_transpose matmuls against an identity,
    packed 7-8 windows per bf16 PSUM bank, one strided evac per pack) into
    15 feature-major pair tiles xT[a] = [128 feats of nodes (a, a+1),
    batch].  No DRAM staging round trip and no DMA-xbar transposes (the
    xbar path runs at ~105 GB/s aggregate and saturates all 16 queues).
    Overlapping adjacent pairs (not just even pairs) pack every target's
    neighbor list into 47 K=128 contraction chunks vs 55 for even-only.
  - L1/L2 run weight-stationary: psum[h, batch] = W.T @ act; relu+bias is
    fused into the PSUM->SBUF evacuation, cost-balanced between ScalarE
    (activation) and VectorE (tensor_scalar) - the only two PSUM readers.
  - The emission is software-pipelined one target deep (L1(t) before
    L2(t-1); L3 groups trail by 2 targets; next tile's transpose packs
    woven between targets) so the in-order PE queue never heads into an
    op whose upstream evacuation is still in flight - this keeps the PE
    ~88% busy and the HAM clock gate warm.
  - L3 runs activation-stationary (lhsT = h2 chunk [128 feats, 128 batch],
    rhs = W3 [128, 64]) so the output lands batch-major in PSUM.  b3 (a
    free-dim bias in this orientation) is added during the PSUM->SBUF
    evacuation against a host-replicated broadcast tile - no K=1 bias
    matmuls on the PE.
  - PSUM: 6 banks rotate L1/L2 accumulators (bufs=3 of [128,1024] fp32),
    2 banks shared by transpose packs and L3 tiles.
  - Output is staged bf16 and stored per L3 group on the GpSimd queue
    (ordered behind the next tile's cast DMAs); the host upcasts to fp32.
"""

import os
import numpy as np
import ml_dtypes

B, J, D, H1, H2 = 65536, 21, 64, 128, 128
NCORES = 8
BC = B // NCORES          # 8192 samples per core
TILE = 1024               # batch tile (2 PSUM banks wide in fp32)
NTILES = BC // TILE       # 8
NCHUNK = TILE // 128      # 8 L3 batch chunks of 128 per tile

FINGER_BASE = [4 * f + 1 for f in range(5)]
NEIGH = {
    6: [[0, 1, 5, 9, 13, 17]],
    5: [[0, 5, 6, 1, 9], [0, 9, 10, 5, 13], [0, 13, 14, 9, 17]],
    4: [[0, 1, 2, 5], [0, 17, 18, 13]],
    3: [r for b in FINGER_BASE for r in ([b, b + 1, b + 2], [b + 1, b + 2, b + 3])],
    2: [[b + 2, b + 3] for b in FINGER_BASE],
}
OUT = {
    6: [0],
    5: [5, 9, 13],
    4: [1, 17],
    3: [j for b in FINGER_BASE for j in (b + 1, b + 2)],
    2: [b + 3 for b in FINGER_BASE],
}
GROUPS = [6, 5, 4, 3, 2]

# target t -> (n, row index within its group, neighbor list)
TARGET = {}
for n in GROUPS:
    for row, t in enumerate(OUT[n]):
        TARGET[t] = (n, row, list(NEIGH[n][row]))

# xT pair tiles: tile `a` holds nodes (a, a+1) feature-major; built by one
# DMA-xbar transpose of xbf columns [64a : 64a+128].
TILE_STARTS = [0, 2, 3, 5, 6, 7, 9, 10, 11, 13, 14, 15, 17, 18, 19]


def node_slots(j):
    """All (tile, half) positions where node j lives."""
    out = []
    if j in TILE_STARTS:
        out.append((j, 0))
    if j - 1 in TILE_STARTS:
        out.append((j - 1, 1))
    return out


def build_chunk_plan():
    """Per target, split neighbor positions into K=128 contraction chunks.

    chunk = dict(tile, slots) with slots = (pos_or_None for half 0,
    pos_or_None for half 1); position i covers W1 rows 64*i : 64*i+64.
    Adjacent neighbors (j, j+1) share a chunk via pair tile j; leftovers
    become half-empty chunks on any tile containing their node (the unused
    64 lhsT rows are zero in the packed W1, so any tile works).
    """
    plan = {}
    for t in range(21):
        n, _, S = TARGET[t]
        best = None
        # brute-force max matching over adjacent pairs (n <= 6)
        import itertools
        idx = list(range(n))
        pairs = [(i, k) for i in idx for k in idx if i < k
                 and abs(S[i] - S[k]) == 1 and min(S[i], S[k]) in TILE_STARTS]

        def search(used, chosen):
            nonlocal best
            cand = [p for p in pairs if not (used & (1 << p[0])) and not (used & (1 << p[1]))]
            if not cand:
                if best is None or len(chosen) > len(best):
                    best = list(chosen)
                return
            for p in cand:
                search(used | (1 << p[0]) | (1 << p[1]), chosen + [p])
            if best is None or len(chosen) > len(best):
                best = list(chosen)

        search(0, [])
        chunks = []
        used = set()
        for i, k in best:
            a, b = S[i], S[k]
            lo = min(a, b)
            pi, pk = (i, k) if a == lo else (k, i)
            chunks.append(dict(tile=lo, slots=(pi, pk)))
            used.update((i, k))
        for i in range(n):
            if i in used:
                continue
            tile_a, half = node_slots(S[i])[0]
            slots = (i, None) if half == 0 else (None, i)
            chunks.append(dict(tile=tile_a, slots=slots))
        plan[t] = chunks
    return plan


CHUNK_PLAN = build_chunk_plan()
TOTAL_CHUNKS = sum(len(v) for v in CHUNK_PLAN.values())

# L3 node groups sharing one PSUM bank (64 fp32 of output each)
L3_GROUPS = [list(range(0, 8)), list(range(8, 16)), list(range(16, 21))]


def pack_weights(inputs):
    """Host-side prep: permute/pack all weights into a handful of flat arrays."""
    bf16 = ml_dtypes.bfloat16
    w1p = np.zeros((128, 128 * TOTAL_CHUNKS), np.float32)
    col = 0
    chunk_cols = {}
    for t in range(21):
        n, row, S = TARGET[t]
        W1 = np.asarray(inputs[f"w1_g{n}"][row], np.float32)  # [n*64, 128]
        for ci, ch in enumerate(CHUNK_PLAN[t]):
            for half, pos in enumerate(ch["slots"]):
                if pos is not None:
                    w1p[64 * half:64 * half + 64, col:col + 128] = \
                        W1[64 * pos:64 * pos + 64]
            chunk_cols[(t, ci)] = col
            col += 128
    w2p = np.zeros((128, 128 * 21), np.float32)
    w3p = np.zeros((128, 64 * 21), np.float32)
    b1p = np.zeros((128, 21), np.float32)
    b2p = np.zeros((128, 21), np.float32)
    b3row = np.zeros(64 * 21, np.float32)
    for t in range(21):
        n, row, _ = TARGET[t]
        w2p[:, 128 * t:128 * (t + 1)] = np.asarray(inputs[f"w2_g{n}"][row])
        w3p[:, 64 * t:64 * (t + 1)] = np.asarray(inputs[f"w3_g{n}"][row])
        b1p[:, t] = np.asarray(inputs[f"b1_g{n}"][row])
        b2p[:, t] = np.asarray(inputs[f"b2_g{n}"][row])
        b3row[64 * t:64 * t + 64] = np.asarray(inputs[f"b3_g{n}"][row])
    b3bc = np.ascontiguousarray(np.broadcast_to(b3row, (128, 64 * 21)))
    return dict(
        w1p=w1p.astype(bf16), w2p=w2p.astype(bf16), w3p=w3p.astype(bf16),
        b1p=b1p, b2p=b2p, b3bc=b3bc,
    ), chunk_cols


def numpy_emulate(inputs, x):
    """Bit-layout-faithful numpy model of what the HW kernel computes (minus
    PSUM rounding): used to validate the chunk plan / packing offline."""
    bf16 = ml_dtypes.bfloat16
    packed, chunk_cols = pack_weights(inputs)
    xb = x.astype(bf16)  # [Bn, 21, 64]
    Bn = x.shape[0]
    xT = {}
    for a in TILE_STARTS:
        xT[a] = np.concatenate([xb[:, a], xb[:, a + 1]], 1).T  # [128, Bn]
    out = np.zeros((Bn, 21, 64), np.float32)
    for t in range(21):
        psum1 = np.zeros((128, Bn), np.float32)
        for ci, ch in enumerate(CHUNK_PLAN[t]):
            col = chunk_cols[(t, ci)]
            lhsT = packed["w1p"][:, col:col + 128].astype(np.float32)
            rhs = xT[ch["tile"]].astype(np.float32)
            psum1 += lhsT.T @ rhs
        h1 = np.maximum(psum1 + packed["b1p"][:, t:t + 1], 0).astype(bf16)
        w2 = packed["w2p"][:, 128 * t:128 * (t + 1)].astype(np.float32)
        psum2 = w2.T @ h1.astype(np.float32)
        h2 = np.maximum(psum2 + packed["b2p"][:, t:t + 1], 0).astype(bf16)
        w3 = packed["w3p"][:, 64 * t:64 * (t + 1)].astype(np.float32)
        b3 = packed["b3bc"][0, 64 * t:64 * t + 64]
        o = (h2.astype(np.float32).T @ w3) + b3[None, :]
        out[:, t] = o.astype(bf16).astype(np.float32)
    return out


# ---------------------------------------------------------------------------
# Bass kernel
# ---------------------------------------------------------------------------

def build_bass_kernel():
    import concourse.bass as bass
    import concourse.tile as tile
    from concourse import bacc, mybir

    bf16 = mybir.dt.bfloat16
    f32 = mybir.dt.float32
    Relu = mybir.ActivationFunctionType.Relu
    Alu = mybir.AluOpType

    from concourse import masks

    nc = bacc.Bacc("TRN2", target_bir_lowering=False, debug=False,
                   num_devices=NCORES)
    x_dram = nc.dram_tensor("x", [BC, J, D], f32, kind="ExternalInput").ap()
    out_dram = nc.dram_tensor("out", [BC, J, D], bf16, kind="ExternalOutput").ap()
    w1_dram = nc.dram_tensor("w1p", [128, 128 * TOTAL_CHUNKS], bf16,
                             kind="ExternalInput").ap()
    w2_dram = nc.dram_tensor("w2p", [128, 128 * 21], bf16, kind="ExternalInput").ap()
    w3_dram = nc.dram_tensor("w3p", [128, 64 * 21], bf16, kind="ExternalInput").ap()
    b1_dram = nc.dram_tensor("b1p", [128, 21], f32, kind="ExternalInput").ap()
    b2_dram = nc.dram_tensor("b2p", [128, 21], f32, kind="ExternalInput").ap()
    b3_dram = nc.dram_tensor("b3bc", [128, 64 * 21], f32, kind="ExternalInput").ap()

    F = J * D  # 1344
    # [128, BC//128, F] views: global batch row = q*128 + p
    x_q = x_dram.rearrange("(q p) t d -> p q (t d)", p=128)
    out_q = out_dram.rearrange("(q p) t d -> p q (t d)", p=128)

    NW = len(TILE_STARTS)  # 15 transpose windows / xT pair tiles
    WIDX = {a: i for i, a in enumerate(TILE_STARTS)}
    # transpose packs: (chunk c, window sublist) -> one bf16 PSUM bank
    PACK_W = [TILE_STARTS[0:8], TILE_STARTS[8:15]]

    with tile.TileContext(nc) as tc:
        with (
            tc.tile_pool(name="wpool", bufs=1) as wpool,
            tc.tile_pool(name="xbp", bufs=2) as xbp,
            tc.tile_pool(name="xtp", bufs=2) as xtp,
            tc.tile_pool(name="actp", bufs=3) as actp,
            tc.tile_pool(name="h2p", bufs=1) as h2p,
            tc.tile_pool(name="stgp", bufs=1) as stgp,
            tc.tile_pool(name="ps12", bufs=3, space="PSUM") as ps12,
            tc.tile_pool(name="scrp", bufs=2, space="PSUM") as scrp,
        ):
            w1s = wpool.tile([128, 128 * TOTAL_CHUNKS], bf16, name="w1s")
            w2s = wpool.tile([128, 128 * 21], bf16, name="w2s")
            w3s = wpool.tile([128, 64 * 21], bf16, name="w3s")
            b1s = wpool.tile([128, 21], f32, name="b1s")
            b2s = wpool.tile([128, 21], f32, name="b2s")
            b3s = wpool.tile([128, 64 * 21], f32, name="b3s")
            ident = wpool.tile([128, 128], bf16, name="ident")
            nc.sync.dma_start(w1s[:], w1_dram)
            nc.sync.dma_start(w2s[:], w2_dram)
            nc.sync.dma_start(w3s[:], w3_dram)
            nc.sync.dma_start(b1s[:], b1_dram)
            nc.sync.dma_start(b2s[:], b2_dram)
            nc.sync.dma_start(b3s[:], b3_dram)
            masks.make_identity(nc, ident[:])

            # explicit PSUM-evac engine balancing: the two PSUM readers
            # (ScalarE 1.2GHz, VectorE 0.96GHz) get ops by tracked cost
            eng_load = [0.0, 0.0]  # [scalar_ns, vector_ns]

            def evac_relu(dst, src, bias_col, ncols):
                if eng_load[0] + (ncols + 520) / 1.2 <= \
                   eng_load[1] + (ncols + 300) / 0.96:
                    eng_load[0] += (ncols + 520) / 1.2
                    nc.scalar.activation(dst, src, Relu, bias=bias_col, scale=1.0)
                else:
                    eng_load[1] += (ncols + 300) / 0.96
                    nc.vector.tensor_scalar(dst, src, bias_col, 0.0,
                                            Alu.add, Alu.max)

            def evac_copy(dst, src, ncols):
                if eng_load[0] + (ncols + 520) / 1.2 <= \
                   eng_load[1] + (ncols + 300) / 0.96:
                    eng_load[0] += (ncols + 520) / 1.2
                    nc.scalar.copy(dst, src)
                else:
                    eng_load[1] += (ncols + 300) / 0.96
                    nc.vector.tensor_copy(dst, src)

            def evac_add(dst, src, bcast, ncols):
                # free-dim bias add: tensor_tensor exists only on VectorE
                eng_load[1] += (ncols + 300) / 0.96
                nc.vector.tensor_tensor(dst, src, bcast, Alu.add)

            # per-tile input tiles (created by casts / first pack)
            xb_tiles = [None] * NTILES   # [128, NCHUNK, F] bf16 batch-major
            xT_tiles = [None] * NTILES   # [128, NW*1024] bf16 feature-major

            def casts(it):
                # 4 SWDGE cast DMAs: x fp32 DRAM -> bf16 SBUF, batch-major
                xb = xbp.tile([128, NCHUNK, F], bf16, tag="xb", name="xb")
                xb_tiles[it] = xb
                for c in range(4):
                    nc.gpsimd.dma_start(
                        xb[:, 2 * c:2 * (c + 1), :],
                        x_q[:, NCHUNK * it + 2 * c:NCHUNK * it + 2 * (c + 1), :])

            def pack(it, pi):
                # PE-transpose one pack: 7-8 [128,128] windows of batch-chunk c
                # into one bf16 PSUM bank, then a single strided evac into the
                # feature-major xT tile.  Half-0 packs (windows 0-7) come
                # first so targets touching only those windows can start
                # after 8 of the 16 prologue packs.
                half, c = divmod(pi, NCHUNK)
                ws = PACK_W[half]
                if pi == 0:
                    xT_tiles[it] = xtp.tile([128, NW * TILE], bf16,
                                            tag="xT", name="xT")
                xb = xb_tiles[it]
                xT3 = xT_tiles[it].rearrange("p (w n) -> p w n", n=TILE)
                pst = scrp.tile([128, 1024], bf16, tag="scr", name="pst")
                for k, a in enumerate(ws):
                    nc.tensor.transpose(pst[:, 128 * k:128 * (k + 1)],
                                        xb[:, c, 64 * a:64 * a + 128],
                                        ident[:])
                L = len(ws)
                w0 = WIDX[ws[0]]
                evac_copy(xT3[:, w0:w0 + L, 128 * c:128 * (c + 1)],
                          pst[:, 0:128 * L].rearrange("p (w n) -> p w n", n=128),
                          128 * L)

            casts(0)
            for pi in range(2 * NCHUNK):
                pack(0, pi)

            for it in range(NTILES):
                if it + 1 < NTILES:
                    casts(it + 1)
                xT3 = xT_tiles[it].rearrange("p (w n) -> p w n", n=TILE)
                stg = stgp.tile([128, NCHUNK * F], bf16, tag="stg", name="stg")
                stg3 = stg.rearrange("p (c f) -> p c f", f=F)
                h2tiles = {}

                def l3_group(grp):
                    # all 8 batch-chunks of one 64*len(grp)-wide output block;
                    # store immediately (gpsimd queue, behind next tile's casts)
                    gw = 64 * len(grp)
                    c0 = 64 * grp[0]
                    for c in range(NCHUNK):
                        psum3 = scrp.tile([128, 512], f32, tag="scr",
                                         name="psum3")
                        for k, t in enumerate(grp):
                            nc.tensor.matmul(
                                psum3[:, 64 * k:64 * (k + 1)],
                                h2tiles[t][:, 128 * c:128 * (c + 1)],
                                w3s[:, 64 * t:64 * (t + 1)],
                                start=True, stop=True,
                                skip_group_check=True)
                        evac_add(stg3[:, c, c0:c0 + gw], psum3[:, 0:gw],
                                 b3s[:, c0:c0 + gw], gw)
                    nc.gpsimd.dma_start(
                        out_q[:, NCHUNK * it:NCHUNK * (it + 1), c0:c0 + gw],
                        stg3[:, :, c0:c0 + gw])

                # software-pipelined emission: the PE stream interleaves
                # independent work (L1 of target t) between each dependent
                # pair (evac1(t-1) -> L2(t-1)), so the in-order PE queue
                # never heads into an op whose upstream evac is still
                # in flight.  L3 groups trail their last target by 2.
                def emit_l1(t):
                    chunks = CHUNK_PLAN[t]
                    psum1 = ps12.tile([128, TILE], f32, tag="ps", name="psum1")
                    for ci, ch in enumerate(chunks):
                        col = CHUNK_COLS[(t, ci)]
                        for h in range(TILE // 512):
                            nc.tensor.matmul(
                                psum1[:, 512 * h:512 * (h + 1)],
                                w1s[:, col:col + 128],
                                xT3[:, WIDX[ch["tile"]], 512 * h:512 * (h + 1)],
                                start=(ci == 0), stop=(ci == len(chunks) - 1),
                                skip_group_check=True)
                    h1 = actp.tile([128, TILE], bf16, tag="h1", name="h1")
                    evac_relu(h1[:], psum1[:], b1s[:, t:t + 1], TILE)
                    return h1

                def emit_l2(t, h1):
                    psum2 = ps12.tile([128, TILE], f32, tag="ps", name="psum2")
                    for h in range(TILE // 512):
                        nc.tensor.matmul(
                            psum2[:, 512 * h:512 * (h + 1)],
                            w2s[:, 128 * t:128 * (t + 1)],
                            h1[:, 512 * h:512 * (h + 1)],
                            start=True, stop=True)
                    h2 = h2p.tile([128, TILE], bf16, tag=f"h2_{t}",
                                  name=f"h2_{t}")
                    evac_relu(h2[:], psum2[:], b2s[:, t:t + 1], TILE)
                    h2tiles[t] = h2

                # half-0-window-only targets (1..7) first so tile 0's
                # L1 can start after 8 of 16 prologue packs; t0 (needing
                # half-1 windows) goes last within its L3 group.  L3 bursts
                # keep v7's structure and trail group completion by 2.
                order = [1, 2, 3, 4, 5, 6, 7, 0] + list(range(8, 21))
                h1_prev = tprev = None
                for k, t in enumerate(order):
                    h1_cur = emit_l1(t)
                    if tprev is not None:
                        emit_l2(tprev, h1_prev)
                    h1_prev, tprev = h1_cur, t
                    # weave next tile's transpose packs between targets
                    if it + 1 < NTILES and k < 2 * NCHUNK:
                        pack(it + 1, k)
                    if k == 9:
                        l3_group(L3_GROUPS[0])
                    elif k == 17:
                        l3_group(L3_GROUPS[1])
                emit_l2(tprev, h1_prev)
                l3_group(L3_GROUPS[2])

    nc.compile()
    return nc


PACKED = None
CHUNK_COLS = None
_NC = None
LAST_RESULT = None


def prepare(inputs):
    """Build (once) the bass module and the per-core input maps."""
    global PACKED, CHUNK_COLS, _NC
    import sys
    if "/opt/trn_rl_repo" not in sys.path:
        sys.path.insert(0, "/opt/trn_rl_repo")
    x = np.ascontiguousarray(np.asarray(inputs["x"], np.float32))
    PACKED, CHUNK_COLS = pack_weights(inputs)
    if _NC is None:
        _NC = build_bass_kernel()
    in_maps = []
    for core in range(NCORES):
        m = dict(PACKED)
        m["x"] = x[core * BC:(core + 1) * BC]
        in_maps.append(m)
    return _NC, in_maps


def kernel(**inputs):
    global LAST_RESULT
    nc, in_maps = prepare(inputs)
    from concourse.bass_utils import run_bass_kernel_spmd
    res = run_bass_kernel_spmd(nc, in_maps, core_ids=list(range(NCORES)),
                               tmpdir=os.environ.get("BASS_TMPDIR"))
    LAST_RESULT = res
    out = np.concatenate([r["out"] for r in res.results], 0)
    return out.reshape(B, J, D).astype(np.float32)
